# revision 1
# baseline (speedup 1.0000x reference)
"""Trainium2 Bass kernel v2 for nn_MetaComprehensiveRegularization.

loss_common  = -sum(zc*zc); loss_special = -sum_v sum_i cos(zc_i, zs_vi).
Data-parallel over N on 8 cores; each core ships per-row stats (zc^2,
dot, zs^2) to the host, which combines in fp64.

vs v1 (74.6us measured):
- DMA stream: fine-grained zs chunks with 0.5MiB zc slices woven in at
  ~20% duty so the ACT engine (squares only; it can only track the
  stream) is never starved by a zc lump; 0.5MiB tail chunks so compute
  drains right behind the last byte.
- Each DMA chunk gets its OWN SBUF slot, and slots are allocated in an
  interleaved order so the chunk being written by DMA is far (in SBUF
  address space) from chunks being read by the engines - avoids the
  SBUF bank conflicts that inflated op durations ~20% when streaming
  into one big tile.
- Engine split from measured contended cadences (DVE ~612ns/block-op,
  ACT ~755ns): DVE = 16 zc^2 + 64 dots, ACT = 64 zs^2.
- 28 sems (v1: 31); ACT accumulates in PSUM (faster READ_ACCUMULATOR),
  one PSUM->SBUF copy at the end.
"""

from contextlib import ExitStack

import numpy as np

N_CORES = 8
N, D, V = 16384, 512, 4
N_LOC = N // N_CORES      # 2048
P = 128
A = 16                    # rows per partition: row = p*A + a

# DMA schedule: ('zc', lo, hi) or (v, lo, hi) in a-block units (0.25 MiB each).
SCHEDULE = [
    (0, 0, 2), ("zc", 0, 2), (0, 2, 4), ("zc", 2, 4), (0, 4, 8),
    ("zc", 4, 6), (0, 8, 12), ("zc", 6, 8), (0, 12, 16), ("zc", 8, 10),
    (1, 0, 4), ("zc", 10, 12), (1, 4, 8), ("zc", 12, 16), (1, 8, 12),
    (1, 12, 16), (2, 0, 4), (2, 4, 8), (2, 8, 12), (2, 12, 16),
    (3, 0, 4), (3, 4, 8), (3, 8, 12), (3, 12, 14), (3, 14, 16),
]

_PROGRAM = None


def _chunk_maps():
    zc_chunk = {}
    zs_chunk = {}
    for i, (kind, lo, hi) in enumerate(SCHEDULE):
        for a in range(lo, hi):
            if kind == "zc":
                zc_chunk[a] = i
            else:
                zs_chunk[(kind, a)] = i
    return zc_chunk, zs_chunk


# squares run on ACT except these (late ones moved to DVE to balance ends)
DVE_SQUARES = set()


def _engine_programs():
    zc_chunk, zs_chunk = _chunk_maps()
    dve = [(zc_chunk[a], ("zc2", a)) for a in range(A)]
    dve += [
        (max(zc_chunk[a], zs_chunk[(v, a)]), ("dot", v, a))
        for v in range(V)
        for a in range(A)
    ]
    dve += [(zs_chunk[va], ("sq",) + va) for va in sorted(DVE_SQUARES)]
    dve.sort(key=lambda x: (x[0], x[1]))
    act = [
        (zs_chunk[(v, a)], ("sq", v, a))
        for v in range(V)
        for a in range(A)
        if (v, a) not in DVE_SQUARES
    ]
    act.sort(key=lambda x: (x[0], x[1]))
    return dve, act


def _build_program():
    import concourse.bacc as bacc
    from concourse import mybir

    f32 = mybir.dt.float32
    nc = bacc.Bacc(
        "TRN2", target_bir_lowering=False, debug=False, num_devices=N_CORES
    )
    zc_t = nc.dram_tensor("zc", [N_LOC, D], f32, kind="ExternalInput")
    zs_t = nc.dram_tensor("zs", [V, N_LOC, D], f32, kind="ExternalInput")
    n_dsq = len(DVE_SQUARES)
    outv_t = nc.dram_tensor("outv", [P, 80 + n_dsq], f32, kind="ExternalOutput")
    outs_t = nc.dram_tensor("outs", [P, 64 - n_dsq], f32, kind="ExternalOutput")

    zc_v = zc_t.ap().rearrange("(p a) d -> p a d", a=A, p=P)
    zs_v = zs_t.ap().rearrange("v (p a) d -> v p a d", a=A, p=P)
    mult = mybir.AluOpType.mult
    Sq = mybir.ActivationFunctionType.Square

    dve_prog, act_prog = _engine_programs()
    nchunks = len(SCHEDULE)

    with ExitStack() as ctx:
        # One SBUF slot per DMA chunk, allocated in stride-2 interleaved
        # order so chunks consecutive in stream time sit ~half the slot
        # space apart in SBUF (no read/write bank conflicts).
        alloc_order = list(range(0, nchunks, 2)) + list(range(1, nchunks, 2))
        slots = {}
        for i in alloc_order:
            kind, lo, hi = SCHEDULE[i]
            slots[i] = ctx.enter_context(
                nc.sbuf_tensor(f"t{i}", [P, hi - lo, D], f32)
            )
        stats_v = ctx.enter_context(nc.sbuf_tensor("sv", [P, 80 + n_dsq], f32))
        stats_s = ctx.enter_context(nc.psum_tensor("ss", [P, 64 - n_dsq], f32))
        ss_sb = ctx.enter_context(nc.sbuf_tensor("ssb", [P, 64 - n_dsq], f32))
        scr = [
            ctx.enter_context(nc.sbuf_tensor(f"scr{i}", [P, D], f32))
            for i in range(4)
        ]
        dummy_s = ctx.enter_context(nc.psum_tensor("ds", [P, 64 - n_dsq], f32))

        dma_sems = [
            ctx.enter_context(nc.semaphore(f"dma{i}")) for i in range(nchunks)
        ]
        sem_v = ctx.enter_context(nc.semaphore("sem_v"))
        sem_s = ctx.enter_context(nc.semaphore("sem_s"))
        sem_out = ctx.enter_context(nc.semaphore("out"))

        zc_chunk, zs_chunk = _chunk_maps()

        def tile(kind, a):
            ci = zc_chunk[a] if kind == "zc" else zs_chunk[(kind, a)]
            lo = SCHEDULE[ci][1]
            return slots[ci].ap()[:, a - lo, :]

        for i, (kind, lo, hi) in enumerate(SCHEDULE):
            src = zc_v[:, lo:hi, :] if kind == "zc" else zs_v[kind][:, lo:hi, :]
            nc.sync.dma_start(out=slots[i].ap(), in_=src).then_inc(dma_sems[i], 16)

        # ---- DVE: 16 zc^2 + 64 dots ----
        waited = set()
        k = 0
        for gate, op in dve_prog:
            if gate not in waited:
                nc.vector.wait_ge(dma_sems[gate], 16)
                waited.add(gate)
            if op[0] == "zc2":
                a = op[1]
                in0 = in1 = tile("zc", a)
                col = a
            elif op[0] == "sq":
                _, v, a = op
                in0 = in1 = tile(v, a)
                col = 80 + sorted(DVE_SQUARES).index((v, a))
            else:
                _, v, a = op
                in0, in1 = tile("zc", a), tile(v, a)
                col = 16 + v * 16 + a
            nc.vector.scalar_tensor_tensor(
                out=scr[k % 4].ap(),
                in0=in0,
                scalar=1.0,
                in1=in1,
                op0=mult,
                op1=mult,
                accum_out=stats_v.ap()[:, col : col + 1],
            ).then_inc(sem_v, 1)
            k += 1

        # ---- ACT: 64 zs^2 ----
        waited_s = set()
        act_col = 0
        for gate, op in act_prog:
            if gate not in waited_s:
                nc.scalar.wait_ge(dma_sems[gate], 16)
                waited_s.add(gate)
            _, v, a = op
            col = act_col
            act_col += 1
            nc.scalar.activation(
                out=dummy_s.ap()[:, col : col + 1].broadcast_to((P, D)),
                in_=tile(v, a),
                func=Sq,
                accum_out=stats_s.ap()[:, col : col + 1],
            ).then_inc(sem_s, 1)

        nc.scalar.wait_ge(sem_s, 64 - n_dsq)
        nc.scalar.copy(out=ss_sb.ap(), in_=stats_s.ap()).then_inc(sem_s, 1)

        nc.sync.wait_ge(sem_v, 80 + n_dsq)
        nc.sync.dma_start(out=outv_t.ap(), in_=stats_v.ap()).then_inc(sem_out, 16)
        nc.sync.wait_ge(sem_s, 65 - n_dsq)
        nc.sync.dma_start(out=outs_t.ap(), in_=ss_sb.ap()).then_inc(sem_out, 16)
        nc.sync.wait_ge(sem_out, 32)

    nc.compile()
    return nc


def _get_program():
    global _PROGRAM
    if _PROGRAM is None:
        _PROGRAM = _build_program()
    return _PROGRAM


def _combine(stats_v: np.ndarray, stats_s: np.ndarray):
    """stats_v: [cores, P, 80+ndsq], stats_s: [cores, P, 64-ndsq] (ACT order)."""
    sv = stats_v.astype(np.float64)
    ss = stats_s.astype(np.float64)
    cn2 = sv[:, :, 0:16]
    dot = sv[:, :, 16:80].reshape(sv.shape[0], P, V, A)
    _, zs_chunk = _chunk_maps()
    act_list = sorted(
        ((zs_chunk[(v, a)], v, a) for v in range(V) for a in range(A)
         if (v, a) not in DVE_SQUARES),
    )
    sn2 = np.empty((sv.shape[0], P, V, A))
    for j, (_, v, a) in enumerate(act_list):
        sn2[:, :, v, a] = ss[:, :, j]
    for j, (v, a) in enumerate(sorted(DVE_SQUARES)):
        sn2[:, :, v, a] = sv[:, :, 80 + j]
    common = cn2.sum()
    eps = 1e-12
    cn = np.maximum(np.sqrt(cn2), eps)
    sn = np.maximum(np.sqrt(sn2), eps)
    special = (dot / (cn[:, :, None, :] * sn)).sum()
    return common, special


def kernel(zc: np.ndarray, zs: np.ndarray) -> np.ndarray:
    from concourse.bass_utils import run_bass_kernel_spmd

    zc = np.ascontiguousarray(np.asarray(zc), dtype=np.float32)
    zs = np.ascontiguousarray(np.asarray(zs), dtype=np.float32)
    assert zc.shape == (N, D) and zs.shape == (V, N, D)

    nc = _get_program()
    in_maps = [
        {
            "zc": np.ascontiguousarray(zc[i * N_LOC : (i + 1) * N_LOC]),
            "zs": np.ascontiguousarray(zs[:, i * N_LOC : (i + 1) * N_LOC]),
        }
        for i in range(N_CORES)
    ]
    res = run_bass_kernel_spmd(nc, in_maps, core_ids=list(range(N_CORES)))
    stats_v = np.stack([r["outv"] for r in res.results])
    stats_s = np.stack([r["outs"] for r in res.results])
    common, special = _combine(stats_v, stats_s)
    return np.asarray([-common, -special], dtype=np.float32)

